# revision 50
# baseline (speedup 1.0000x reference)
"""Trainium2 Bass kernel for nn_DistillingLayer: per-channel shared-weight
Conv1d(k=3, stride=2, pad=1) + ELU + MaxPool1d(k=3, stride=2, pad=1) over
x:(16, 4096, 512) f32 -> out:(16, 1024, 512) f32.

Strategy
--------
- Data-parallel over batch: 8 cores x 2 batches each. No communication.
- Layout: L lives in the SBUF *free* dimension. Each partition owns S=32
  consecutive L-rows (x D=512 channels) plus a 3-row halo loaded with
  overlap from HBM, so the whole conv+pool dataflow stays per-partition
  local. One tile per batch (128 * 32 = 4096 rows); halo re-read is 3/32.
- The input is zero-padded by 3 L-rows on the host (uniform full-128
  DMAs + free conv left-padding).
- f32 -> bf16 cast happens INSIDE the input DMA (SWDGE cast): HBM traffic
  is unchanged but DVE ops get their 2x bf16 perf modes. End-to-end
  bf16 error is ~6e-3 absmax-scaled, under the 2e-2 gate.
- Per conv segment c = w0*A + w1*O + w2*A' + bias (A/O/A' = even/odd/
  even+1 strided row views), work is balanced across ScalarE and DVE:
    ScalarE: tap0 = Copy(w0*A + bias); for the four mid-size segments
             also T2 = Copy(w2*A'); later Exp for the ELU.
    DVE:     tap1 accumulate via scalar_tensor_tensor (1x, but equal to
             a 2x scale + 2x add pair in DVE time with fewer instrs);
             tap2 via stt or a 2x tensor_tensor add of T2; pool maxes
             (tt, 2x); the ELU finish.
- ELU is monotonic so it commutes with maxpool: pool pre-activation, then
  ELU only on the pooled rows: ELU(v) = max(min(exp(v),1) - 1, v) as one
  ScalarE Exp, a DVE tensor_scalar (min,add) and a 2x bf16 tensor_tensor
  max (exact in bf16). The output DMA casts bf16->f32 (SWDGE, gpsimd
  ring): emitted behind the 12 input chunk DMAs it executes only after
  all reads have drained, so stores never steal read line rate from the
  SDMA engines (this also collapsed most core-to-core variance).
- All 12 input chunk DMAs are emitted first on the gpsimd (SWDGE) queue:
  HBM reads stream back-to-back from t=0 at the SDMA line rate.
- The (tile, segment) work stream is software-pipelined with a 2-segment
  ELU lag, and every emission carries a tile_wait_until hint with the
  DMA-line-rate estimate of its chunk landing time — without these the
  Tile list-scheduler misorders chunk-gated ScalarE ops ahead of ready
  work and idles both engines.
- Weights/bias are baked as immediates; the compiled module is cached per
  (w, b) value.

Toolchain workaround (see inline comment): a BIR post-pass splits
multi-wait instructions - this walrus build allows one sync wait per
instruction.
"""

import json as _json
import os
import sys

import numpy as np

for _p in ("/opt/trn_rl_repo", "/root/.axon_site/_ro/trn_rl_repo"):
    if os.path.isdir(_p) and _p not in sys.path:
        sys.path.append(_p)

import concourse.bass as bass
import concourse.bass2jax as bass2jax
import concourse.bass_utils as bass_utils
import concourse.mybir as mybir
from concourse.bass_utils import run_bass_kernel_spmd
from concourse.tile import TileContext

# ---------------------------------------------------------------------------
# REQUIRED workaround: this container's walrus build rejects instructions
# carrying more than one sync wait ("Too many sync wait commands" in
# setupSyncWait). Tile's scheduler freely attaches several waits to one
# instruction, so post-process the BIR JSON before compile: hoist all but the
# last wait onto same-engine NoOps inserted just before the instruction
# (per-engine program order makes sequential waits equivalent to a
# multi-wait).
# ---------------------------------------------------------------------------

_orig_compile_bir_kernel = bass_utils.compile_bir_kernel


def _split_multi_waits(bir_json: bytes) -> bytes:
    j = _json.loads(bir_json)
    ctr = 0
    changed = False
    for fn in j["functions"]:
        for bb in fn["blocks"]:
            out = []
            for ins in bb["instructions"]:
                si = ins.get("sync_info")
                waits = (si.get("on_wait") or []) if si else []
                if len(waits) > 1:
                    changed = True
                    for w in waits[:-1]:
                        ctr += 1
                        out.append(
                            {
                                "debug": ins.get("debug", 0),
                                "engine": ins["engine"],
                                "ins": [],
                                "outs": [],
                                "name": f"waitsplit-{ctr}",
                                "opcode": "NoOp",
                                "text_hint": "waitsplit",
                                "sync_info": {"on_update": [], "on_wait": [w]},
                            }
                        )
                    si["on_wait"] = [waits[-1]]
                out.append(ins)
            bb["instructions"] = out
    if not changed:
        return bir_json
    return _json.dumps(j).encode()


def _patched_compile_bir_kernel(bir_json, tmpdir, neff_name="file.neff"):
    return _orig_compile_bir_kernel(_split_multi_waits(bir_json), tmpdir, neff_name)


bass_utils.compile_bir_kernel = _patched_compile_bir_kernel
bass2jax.compile_bir_kernel = _patched_compile_bir_kernel

# The first TileContext exit barrier's per-engine drains are redundant (the
# tail waits already cover all completions); use the cheap sequencer-level
# variant there. The SECOND barrier stays full — its drains restore
# engine/queue state so the loaded NEFF can re-execute.
try:
    from concourse.vector_clock import ScopedClock as _ScopedClock

    def _tail_drain_and_barrier(self, tick_clock, wait_clock):
        drain_inst = self.nc.sync.drain()
        wait_clock.add_sem_waits(
            drain_inst.ins, _ScopedClock({None: tick_clock.global_clock})
        )
        self.nc.all_engine_barrier(sem_only=True)
        assert self.sems is not None
        popped = self.nc._tile_sem_poison_stack.pop()
        assert popped is self._sem_poison
        self.nc.clear_and_free_semaphores(list(self.sems.allocated().values()))
        self.nc.all_engine_barrier()

    TileContext._drain_and_barrier = _tail_drain_and_barrier
except Exception:
    pass

# ---------------------------------------------------------------------------

N_CORES = 8
B, L, D = 16, 4096, 512
BPC = B // N_CORES  # batches per core
LC = L // 2         # conv output length
LP = LC // 2        # pool output length

St = 32             # L-rows per partition per tile (one tile per batch)
XR = St + 3         # x rows held per partition (3-row halo)
Q = St // 2 + 1     # conv rows per partition (incl. 1 halo row)
Jt = St // 4        # pool-output rows per partition

F32 = mybir.dt.float32
BF16 = mybir.dt.bfloat16
ALU = mybir.AluOpType
AF = mybir.ActivationFunctionType

# input chunks (local x rows) and the conv/pool segments they unlock:
# conv seg (qa,qb) taps local x rows [2qa, 2qb]; pool seg (ja,jb) reads
# conv rows [2ja, 2jb]. The last chunk/segment pair is kept small so the
# serial chain after the final input chunk lands is short.
CHUNKS = [(0, 4), (4, 11), (11, 18), (18, 25), (25, 31), (31, 35)]
CONV_SEGS = [(0, 1), (1, 5), (5, 8), (8, 12), (12, 15), (15, 17)]
# pool segs emitted after each conv seg (pool (ja,jb) needs conv q<=2jb)
POOL_LISTS = [[], [(0, 2)], [(2, 3)], [(3, 5)], [(5, 7)], [(7, 8)]]
# conv segs whose w2 tap is accumulated via a ScalarE-scaled temp + DVE
# tensor_tensor add (2x) instead of a DVE stt (1x) — shifts ~7us of DVE
# work per core onto the less-loaded ScalarE. ACT_TAP1_SEGS does the same
# for the w1 tap of the largest segment.
ACT_TAP2_SEGS = {1, 2, 3, 4}
ACT_TAP1_SEGS = {3}
# segs whose tap0 runs on DVE (tensor_scalar, 2x) instead of ScalarE:
# for the first segments the chunk->ScalarE-copy->DVE chain would
# otherwise leave DVE idle right after a chunk lands.
DVE_TAP0_SEGS = {0, 1}

_cache: dict = {}

# Exposed for test harnesses: the BassKernelResults of the last run.
LAST_RESULT = None


def _build(w0: float, w1: float, w2: float, bias: float) -> bass.Bass:
    nc = bass.Bass()
    # x is host-padded with 3 zero rows at the front of L: padded row r
    # holds true row r-3 (see module docstring).
    x = nc.dram_tensor("x", [BPC, L + 3, D], F32, kind="ExternalInput")
    y = nc.dram_tensor("y", [BPC, LP, D], F32, kind="ExternalOutput")

    xrow = D              # elements per L-row
    xbat = (L + 3) * D    # elements per (padded) input batch
    ybat = LP * D

    with TileContext(nc) as tc:
        with (
            tc.tile_pool(name="xp", bufs=2) as xp,
            tc.tile_pool(name="yp", bufs=2) as yp,
            tc.tile_pool(name="pp", bufs=2) as pp,
            tc.tile_pool(name="tp", bufs=4) as tp,
            tc.tile_pool(name="ep", bufs=4) as ep,
            tc.tile_pool(name="rp", bufs=10) as rp,
        ):
            # ---- all input DMAs first on the gpsimd (SWDGE) queue so HBM
            # reads stream back-to-back from t=0.
            Xs = []
            for b in range(BPC):
                X = xp.tile([128, XR * D], BF16)
                for (r0, r1) in CHUNKS:
                    nc.gpsimd.dma_start(
                        out=X[:, r0 * D : r1 * D],
                        in_=bass.AP(
                            x,
                            b * xbat + r0 * xrow,
                            [[St * xrow, 128], [1, (r1 - r0) * xrow]],
                        ),
                    )
                Xs.append(X)

            tiles: dict = {}

            def emit_conv_pool(b, s):
                Xv, Y, y3, P, p3 = tiles[b]
                qa, qb = CONV_SEGS[s]
                # conv taps: A = x[2q], O = x[2q+1], A' = x[2q+2]
                ya = Xv[:, 2 * qa : 2 * qb - 1 : 2, :]
                yb = Xv[:, 2 * qa + 1 : 2 * qb : 2, :]
                yc = Xv[:, 2 * qa + 2 : 2 * qb + 1 : 2, :]
                ys = y3[:, qa:qb, :]
                # ScalarE: tap0 (+bias); DVE: the accumulates
                # (scalar_tensor_tensor is 1x, but a 2x scale + 2x add
                # pair costs the same DVE time and more instructions).
                if s in DVE_TAP0_SEGS:
                    nc.vector.tensor_scalar(
                        ys, ya, w0, bias, op0=ALU.mult, op1=ALU.add
                    )
                else:
                    nc.scalar.activation(ys, ya, AF.Copy, bias=bias, scale=w0)
                if s in ACT_TAP1_SEGS:
                    T1 = tp.tile([128, (qb - qa) * D], BF16)
                    t13 = T1[:, :].rearrange("p (q d) -> p q d", d=D)
                    nc.scalar.activation(t13, yb, AF.Copy, scale=w1)
                    nc.vector.tensor_tensor(
                        Y[:, qa * D : qb * D],
                        Y[:, qa * D : qb * D],
                        T1[:, :],
                        op=ALU.add,
                    )
                else:
                    nc.vector.scalar_tensor_tensor(
                        ys, yb, w1, ys, op0=ALU.mult, op1=ALU.add
                    )
                if s in ACT_TAP2_SEGS:
                    T2 = tp.tile([128, (qb - qa) * D], BF16)
                    t23 = T2[:, :].rearrange("p (q d) -> p q d", d=D)
                    nc.scalar.activation(t23, yc, AF.Copy, scale=w2)
                    nc.vector.tensor_tensor(
                        Y[:, qa * D : qb * D],
                        Y[:, qa * D : qb * D],
                        T2[:, :],
                        op=ALU.add,
                    )
                else:
                    nc.vector.scalar_tensor_tensor(
                        ys, yc, w2, ys, op0=ALU.mult, op1=ALU.add
                    )
                if s == 0:
                    # left pool pad: c[-1] = -inf (partition 0 only)
                    nc.vector.memset(Y[0:1, 0:D], float("-inf"))

                # maxpool (pre-activation; ELU is monotonic):
                # pool[j] = max(Y[2j], Y[2j+1], Y[2j+2])
                for (ja, jb) in POOL_LISTS[s]:
                    ps = p3[:, ja:jb, :]
                    nc.vector.tensor_tensor(
                        ps,
                        y3[:, 2 * ja : 2 * jb - 1 : 2, :],
                        y3[:, 2 * ja + 1 : 2 * jb : 2, :],
                        op=ALU.max,
                    )
                    nc.vector.tensor_tensor(
                        ps, ps, y3[:, 2 * ja + 2 : 2 * jb + 1 : 2, :], op=ALU.max
                    )

            def emit_elu(b, ja, jb):
                # ELU(v) = max(min(exp(v),1) - 1, v): one ScalarE Exp, a
                # 2x DVE tensor_scalar (min,add) and a 2x bf16
                # tensor_tensor max (exact in bf16). The output DMA casts
                # bf16->f32 (SWDGE): on the gpsimd ring it queues behind
                # all input reads, so writes never steal read line rate.
                _, _, _, P, _ = tiles[b]
                ps = P[:, ja * D : jb * D]
                E = ep.tile([128, (jb - ja) * D], BF16)
                nc.scalar.activation(E[:, :], ps, AF.Exp)
                nc.vector.tensor_scalar(
                    E[:, :], E[:, :], 1.0, -1.0, op0=ALU.min, op1=ALU.add
                )
                R = rp.tile([128, (jb - ja) * D], BF16)
                nc.vector.tensor_tensor(R[:, :], E[:, :], ps, op=ALU.max)
                nc.gpsimd.dma_start(
                    out=bass.AP(
                        y,
                        b * ybat + ja * xrow,
                        [[Jt * D, 128], [1, (jb - ja) * xrow]],
                    ),
                    in_=R[:, :],
                )

            # Global (tile, seg) stream with the ELU of segment i-2 emitted
            # after the conv+pool of segment i: software-pipelines across
            # the tile boundary so neither engine's program order stalls
            # the other tile's work.
            #
            # tile_wait_until injects estimated ready-times (from the DMA
            # line-rate model: chunks land sequentially on the gpsimd ring
            # at ~27 GiB/s/engine) — without these the Tile list-scheduler
            # underestimates chunk landing times and orders chunk-gated
            # ScalarE copies ahead of ready ELU work, idling engines.
            ch_ready = [
                [11.0, 15.0, 19.0, 23.0, 27.0, 29.0],
                [32.0, 36.0, 40.0, 44.0, 47.0, 50.0],
            ]
            stream = [
                (b, s) for b in range(BPC) for s in range(len(CONV_SEGS))
            ]
            pend: list = []  # (emit_index, b, ja, jb)
            for i, (b, s) in enumerate(stream):
                if s == 0:
                    Xv = Xs[b][:, :].rearrange("p (r d) -> p r d", d=D)
                    Y = yp.tile([128, Q * D], BF16)
                    y3 = Y[:, :].rearrange("p (q d) -> p q d", d=D)
                    P = pp.tile([128, Jt * D], BF16)
                    p3 = P[:, :].rearrange("p (j d) -> p j d", d=D)
                    tiles[b] = (Xv, Y, y3, P, p3)
                with tc.tile_wait_until(ch_ready[b][s] / 1000.0):
                    emit_conv_pool(b, s)
                for seg in POOL_LISTS[s]:
                    pend.append((i, b, seg[0], seg[1], ch_ready[b][s]))
                while pend and pend[0][0] <= i - 2:
                    _, pb, ja, jb, rdy = pend.pop(0)
                    with tc.tile_wait_until((rdy + 4.0) / 1000.0):
                        emit_elu(pb, ja, jb)
            for _, pb, ja, jb, rdy in pend:
                with tc.tile_wait_until((rdy + 4.0) / 1000.0):
                    emit_elu(pb, ja, jb)
    return nc


def kernel(x: np.ndarray, w: np.ndarray, b: np.ndarray) -> np.ndarray:
    global LAST_RESULT
    w = np.asarray(w, dtype=np.float32)
    bb = np.asarray(b, dtype=np.float32)
    key = (float(w[0]), float(w[1]), float(w[2]), float(bb[0]))
    if key not in _cache:
        _cache[key] = _build(*key)
    nc = _cache[key]

    x = np.asarray(x, dtype=np.float32)
    assert x.shape == (B, L, D), x.shape
    xpad = np.zeros((B, L + 3, D), dtype=np.float32)
    xpad[:, 3:, :] = x
    in_maps = [
        {"x": np.ascontiguousarray(xpad[c * BPC : (c + 1) * BPC])}
        for c in range(N_CORES)
    ]
    res = run_bass_kernel_spmd(nc, in_maps, core_ids=list(range(N_CORES)))
    LAST_RESULT = res
    return np.concatenate([r["y"] for r in res.results], axis=0)
